# revision 2
# baseline (speedup 1.0000x reference)
"""Gated-attention kernel for 8 TRN2 NeuronCores.

Problem: out, p = attention(q,k,v) with per-head multiplicative gate
softplus(dm @ w_w + b_w) and additive bias (dm @ w_b + b_b) on the scores,
where dm is built from tiny positional features (rank-2 structure per head).

Sharding: B*H = 32 (batch, head) pairs -> 4 per core (core c gets batch
c//4, heads 4*(c%4) .. 4*(c%4)+3).  All per-head gate/bias maps decompose:
    garg[k,q] = ssum[k]*aw[q] + cw[k]   (folded into one ACT exp pass)
    barg[k,q] = ssum[k]*ab[q] + cb[k]   (folded into scalar_tensor_tensor
                                         + exp bias)
Device computes S^T[k,q] tiles (fp32r matmuls), softplus via exp+ln (one
ACT table set, no switches), softmax denominator via ones-matmul on PE,
P@V directly from the [k,q] layout (no transposes anywhere; q/k arrive
pre-transposed from the host).  Outputs are pT = p^T and outT = out^T;
the host transposes back while unsharding.
"""
import sys
import os

sys.path.insert(0, "/opt/trn_rl_repo")

import numpy as np
import ml_dtypes

import concourse.bass as bass
import concourse.tile as tile
from concourse import mybir
from concourse.bass_utils import run_bass_kernel_spmd

F32 = mybir.dt.float32
F32R = mybir.dt.float32r
BF16 = mybir.dt.bfloat16
AF = mybir.ActivationFunctionType
ALU = mybir.AluOpType

B, H, N, DH, DD = 2, 16, 1024, 128, 8
NC = 8               # cores
J = (B * H) // NC    # bh units per core = 4
NT = N // 128        # 8 k-tiles
HPC = H // (NC // B)  # heads per core = 4


def _softplus_np(x):
    return np.log1p(np.exp(-np.abs(x))) + np.maximum(x, 0)


def build_kernel():
    nc = bass.Bass()
    qT = nc.declare_dram_parameter("qT", [J, DH, N], F32R, isOutput=False)
    kT = nc.declare_dram_parameter("kT", [J, DH, N], F32R, isOutput=False)
    v = nc.declare_dram_parameter("v", [J, N, DH], BF16, isOutput=False)
    ssum_col = nc.declare_dram_parameter("ssum_col", [N, 1], F32, isOutput=False)
    cw_col = nc.declare_dram_parameter("cw_col", [J, N, 1], F32, isOutput=False)
    cb_col = nc.declare_dram_parameter("cb_col", [J, N, 1], F32, isOutput=False)
    aw_row = nc.declare_dram_parameter("aw_row", [J, 1, N], F32, isOutput=False)
    ab_row = nc.declare_dram_parameter("ab_row", [J, 1, N], F32, isOutput=False)
    pT_out = nc.declare_dram_parameter("pT", [J, N, N], F32, isOutput=True)
    outT_out = nc.declare_dram_parameter("outT", [J, DH, N], F32, isOutput=True)
    recip_dram = nc.dram_tensor("recip_scratch", [J, 1, N], F32)

    with tile.TileContext(nc) as tc:
        with (
            tc.tile_pool(name="const", bufs=1) as cpool,
            tc.tile_pool(name="inp", bufs=2) as ipool,
            tc.tile_pool(name="pall", bufs=2) as ppool,
            tc.tile_pool(name="work", bufs=2) as wpool,
            tc.tile_pool(name="pn", bufs=3) as npool,
            tc.tile_pool(name="sps", bufs=2, space="PSUM") as spool,
            tc.tile_pool(name="accps", bufs=1, space="PSUM") as apool,
        ):
            ssum_s = cpool.tile([128, NT], F32)
            nc.sync.dma_start(ssum_s[:], ssum_col.rearrange("(t p) o -> p (t o)", p=128))
            ones_bf = cpool.tile([128, 1], BF16)
            nc.vector.memset(ones_bf[:], 1.0)

            for j in range(J):
                qT_s = ipool.tile([DH, N], F32R, tag="qT")
                nc.sync.dma_start(qT_s[:], qT[j])
                kT_s = ipool.tile([DH, N], F32R, tag="kT")
                nc.sync.dma_start(kT_s[:], kT[j])
                v_s = ipool.tile([128, NT, DH], BF16, tag="v")
                nc.sync.dma_start(v_s[:], v[j].rearrange("(t p) d -> p t d", p=128))
                cw_s = ipool.tile([128, NT], F32, tag="cw")
                nc.sync.dma_start(cw_s[:], cw_col[j].rearrange("(t p) o -> p (t o)", p=128))
                cb_s = ipool.tile([128, NT], F32, tag="cb")
                nc.sync.dma_start(cb_s[:], cb_col[j].rearrange("(t p) o -> p (t o)", p=128))
                awb_s = ipool.tile([128, N], F32, tag="awb")
                nc.sync.dma_start(awb_s[:], aw_row[j].broadcast_to([128, N]))
                abb_s = ipool.tile([128, N], F32, tag="abb")
                nc.sync.dma_start(abb_s[:], ab_row[j].broadcast_to([128, N]))

                P_all = ppool.tile([128, NT, N], BF16, tag="P")
                outU = apool.tile([128, N], F32, tag="outU")
                den = apool.tile([1, N], F32, tag="den")

                for t in range(NT):
                    S_ps = spool.tile([128, N], F32, tag="S")
                    for h in range(2):
                        nc.tensor.matmul(S_ps[:, h * 512:(h + 1) * 512],
                                         lhsT=kT_s[:, t * 128:(t + 1) * 128],
                                         rhs=qT_s[:, h * 512:(h + 1) * 512],
                                         start=True, stop=True)
                    # u = exp(ssum[k]*aw[q] + cw[k]); G = softplus = ln(1+u)
                    u = wpool.tile([128, N], F32, tag="u")
                    nc.scalar.activation(u[:], awb_s[:], AF.Exp,
                                         scale=ssum_s[:, t:t + 1], bias=cw_s[:, t:t + 1])
                    G = wpool.tile([128, N], F32, tag="G")
                    nc.scalar.activation(G[:], u[:], AF.Ln, bias=1.0, scale=1.0)
                    T = wpool.tile([128, N], F32, tag="T")
                    nc.vector.tensor_mul(T[:], S_ps[:], G[:])
                    S2 = wpool.tile([128, N], F32, tag="S2")
                    nc.vector.scalar_tensor_tensor(
                        out=S2[:], in0=abb_s[:], scalar=ssum_s[:, t:t + 1], in1=T[:],
                        op0=ALU.mult, op1=ALU.add)
                    Pt = P_all[:, t, :]
                    nc.scalar.activation(Pt, S2[:], AF.Exp,
                                         bias=cb_s[:, t:t + 1], scale=1.0)
                    for h in range(2):
                        nc.tensor.matmul(outU[:, h * 512:(h + 1) * 512],
                                         lhsT=v_s[:, t, :],
                                         rhs=Pt[:, h * 512:(h + 1) * 512],
                                         start=(t == 0), stop=(t == NT - 1))
                        nc.tensor.matmul(den[0:1, h * 512:(h + 1) * 512],
                                         lhsT=ones_bf[:, 0:1],
                                         rhs=Pt[:, h * 512:(h + 1) * 512],
                                         start=(t == 0), stop=(t == NT - 1))

                recip_s = wpool.tile([1, N], F32, tag="recip")
                nc.vector.reciprocal(recip_s[:], den[:])
                nc.sync.dma_start(recip_dram[j], recip_s[:])
                recipb_s = wpool.tile([128, N], F32, tag="recipb")
                nc.sync.dma_start(recipb_s[:], recip_dram[j].broadcast_to([128, N]))

                for t in range(NT):
                    Pn = npool.tile([128, N], F32, tag="Pn")
                    nc.vector.tensor_mul(Pn[:], P_all[:, t, :], recipb_s[:])
                    nc.sync.dma_start(pT_out[j, t * 128:(t + 1) * 128, :], Pn[:])
                outT_s = npool.tile([128, N], F32, tag="outTs")
                nc.vector.tensor_mul(outT_s[:], outU[:], recipb_s[:])
                nc.sync.dma_start(outT_out[j], outT_s[:])

    from bir_fixup_inline import split_multi_waits
    split_multi_waits(nc)
    return nc


# --- self-contained copy of the BIR wait-split post-pass ---
_fixup_src = '''
from concourse import mybir

_ctr = [0]


def split_multi_waits(nc):
    for fn in nc.m.functions:
        for bb in fn.blocks:
            new = []
            changed = False
            for inst in bb.instructions:
                si = inst.sync_info
                if si is not None and si.on_wait and len(si.on_wait) > 1:
                    waits = list(si.on_wait)
                    for w in waits[:-1]:
                        nop = mybir.InstNoOp(
                            name="I-wsplit-%d" % _ctr[0], ins=[], outs=[])
                        _ctr[0] += 1
                        nop.engine = inst.engine
                        nop.sync_info = mybir.SyncInfo(on_wait=[w], on_update=[])
                        new.append(nop)
                    inst.sync_info = mybir.SyncInfo(
                        on_wait=[waits[-1]], on_update=list(si.on_update))
                    changed = True
                new.append(inst)
            if changed:
                bb.instructions = new
'''
import types

_fixup_mod = types.ModuleType("bir_fixup_inline")
exec(_fixup_src, _fixup_mod.__dict__)
sys.modules["bir_fixup_inline"] = _fixup_mod

_NC_CACHE = {}


def _get_nc():
    if "nc" not in _NC_CACHE:
        _NC_CACHE["nc"] = build_kernel()
    return _NC_CACHE["nc"]


def kernel(q, k, v, c, d_q, d_k_top, d_k_bot, d_k_score, w_w, b_w, w_b, b_b,
           _trace=False, _trace_kwargs=None):
    q = np.asarray(q, np.float32)
    k = np.asarray(k, np.float32)
    v = np.asarray(v, np.float32)
    d_q = np.asarray(d_q, np.float32)
    d_k_top = np.asarray(d_k_top, np.float32)
    d_k_bot = np.asarray(d_k_bot, np.float32)
    d_k_score = np.asarray(d_k_score, np.float32)
    w_w = np.asarray(w_w, np.float32)
    b_w = np.asarray(b_w, np.float32)
    w_b = np.asarray(w_b, np.float32)
    b_b = np.asarray(b_b, np.float32)

    scale = np.float32(1.0 / np.sqrt(DH))
    # host precompute (tiny)
    s0 = d_k_score[:, :, 0, 0]                     # [B,N]
    s1 = d_k_score[:, :, 1, 0]
    ssum = s0 + s1                                 # [B,N]
    dkc = d_k_top * s0[..., None] + d_k_bot * s1[..., None]   # [B,N,DD]
    aw = d_q @ w_w                                 # [B,N,H] (q-side gate vec)
    ab = d_q @ w_b
    cw = b_w[None, None, :] - dkc @ w_w            # [B,N,H] (k-side gate vec)
    cb = b_b[None, None, :] - dkc @ w_b

    qs = (q * scale).transpose(0, 1, 3, 2)         # [B,H,DH,N]
    ks = k.transpose(0, 1, 3, 2)

    in_maps = []
    for core in range(NC):
        b = core // (NC // B)
        h0 = (core % (NC // B)) * HPC
        hs = slice(h0, h0 + HPC)
        in_maps.append({
            "qT": np.ascontiguousarray(qs[b, hs]),
            "kT": np.ascontiguousarray(ks[b, hs]),
            "v": np.ascontiguousarray(v[b, hs]).astype(ml_dtypes.bfloat16),
            "ssum_col": np.ascontiguousarray(ssum[b][:, None]),
            "cw_col": np.ascontiguousarray(cw[b, :, hs].T[:, :, None]),
            "cb_col": np.ascontiguousarray(cb[b, :, hs].T[:, :, None]),
            "aw_row": np.ascontiguousarray(aw[b, :, hs].T[:, None, :]),
            "ab_row": np.ascontiguousarray(ab[b, :, hs].T[:, None, :]),
        })

    nc = _get_nc()
    res = run_bass_kernel_spmd(
        nc, in_maps, core_ids=list(range(NC)),
        trace=_trace, **(_trace_kwargs or {}))

    p_full = np.empty((B, H, N, N), np.float32)
    out_full = np.empty((B, H, N, DH), np.float32)
    for core in range(NC):
        b = core // (NC // B)
        h0 = (core % (NC // B)) * HPC
        r = res.results[core]
        for jj in range(J):
            p_full[b, h0 + jj] = r["pT"][jj].T
            out_full[b, h0 + jj] = r["outT"][jj].T
    kernel.last_results = res
    return out_full, p_full
